# revision 14
# baseline (speedup 1.0000x reference)
"""Dual-branch attention (shared attn weights, se/de value branches) on 8 TRN2 cores.

Sharding: 2 batches x 16 heads = 32 (b,h) pairs; core i owns batch i//4 and
heads [4*(i%4), 4*(i%4)+4) (128 feature channels). Activations are passed
pre-transposed ([C, N]) and in bf16 so the per-core kernel needs no on-chip
transposes. Each core computes its heads' attention for both value branches
and a row-sharded partial of the output projections; the host sums the 4
partials per batch and adds the biases.

v3 (this file), building on v2's wide-exp/deferred-normalize structure:
  - prologue: k-proj PSUM moved OFF the score banks (ps_p/ps_o/ps_d) so the
    first score matmuls only wait on the qtFull + kTz(h0,j0) casts; input
    DMAs are issued from the idle GpSimd queue (fast DGE dispatch) with sT
    before dT; kTz/vpk memsets run on GpSimd instead of clogging the DVE
    queue ahead of the kTz casts.  First exp ~39us -> ~18us.
  - reciprocals use the custom-DVE reciprocal_approx_fast (0.67us vs 4.3us;
    verified ~51 ULP on HW) and the last block's LN+EXP trick is gone.
  - PV matmuls ride a global pending queue lagging the exp stream by 2
    groups (6 during the first iteration so the v-projection, which needs
    the late-arriving dT chunks, never stalls the in-order PE queue), so
    h/j boundaries no longer bunch PV work between exps.
  - the 2-per-group scratch matmuls are dropped (PE is ~90% busy without
    them; HAM only needs the warmup burst + tail fillers).
"""

from collections import deque
from contextlib import ExitStack

import numpy as np
import ml_dtypes

import concourse.bass as bass
import concourse.mybir as mybir
import concourse.tile as tile
from concourse import bacc
from concourse.bass import ts, ds
from concourse.bass_utils import run_bass_kernel_spmd

B, N, C, H, D = 2, 2048, 512, 16, 32
SCALE = D ** -0.5
P = 128
CJ = C // P      # 4 contraction chunks for the projections
NJ = 4           # q blocks of 512
KJ = N // NJ     # 512
NK = N // P      # 16 k chunks of 128
NG = NK // 2     # 8 score groups of 2 k-chunks per (j,h)
HL = 4           # heads per core
F = HL * D       # 128 local feature channels
VW = 2 * D + 1   # per-head vpack width: [v_se | v_de | ones]

BF16 = mybir.dt.bfloat16
F32 = mybir.dt.float32
NPBF16 = ml_dtypes.bfloat16


def build_nc():
    import os
    GPDMA = os.environ.get("KRN_GPDMA", "1") == "1"
    GPMEMSET = os.environ.get("KRN_GPMEMSET", "1") == "1"
    RECIPFAST = os.environ.get("KRN_RECIPFAST", "1") == "1"
    nc = bacc.Bacc("TRN2", target_bir_lowering=False, debug=False, num_devices=8)

    sT = nc.dram_tensor("sT", [C, N], BF16, kind="ExternalInput").ap()
    dT = nc.dram_tensor("dT", [C, N], BF16, kind="ExternalInput").ap()
    wq = nc.dram_tensor("wq", [C, F], BF16, kind="ExternalInput").ap()
    wk = nc.dram_tensor("wk", [C, F], BF16, kind="ExternalInput").ap()
    wvs = nc.dram_tensor("wvs", [C, F], BF16, kind="ExternalInput").ap()
    wvd = nc.dram_tensor("wvd", [C, F], BF16, kind="ExternalInput").ap()
    wps = nc.dram_tensor("wps", [F, C], BF16, kind="ExternalInput").ap()
    wpd = nc.dram_tensor("wpd", [F, C], BF16, kind="ExternalInput").ap()
    # packed output: [branch, partition, n-chunk, C] bf16 partials; the host
    # transposes back to [branch, N, C] and sums partials in fp32.
    out = nc.dram_tensor("out", [2, P, NK, C], BF16, kind="ExternalOutput").ap()

    EXP = mybir.ActivationFunctionType.Exp
    MUL = mybir.AluOpType.mult

    with ExitStack() as ctx:
        tc = ctx.enter_context(tile.TileContext(nc))
        consts = ctx.enter_context(tc.tile_pool(name="consts", bufs=1))
        ppool = ctx.enter_context(tc.tile_pool(name="probs", bufs=12))
        opool = ctx.enter_context(tc.tile_pool(name="opool", bufs=6))
        rpool = ctx.enter_context(tc.tile_pool(name="rpool", bufs=8))
        spool = ctx.enter_context(tc.tile_pool(name="spool", bufs=4))
        ps_sc = ctx.enter_context(tc.tile_pool(name="ps_sc", bufs=2, space="PSUM"))
        ps_o = ctx.enter_context(tc.tile_pool(name="ps_o", bufs=1, space="PSUM"))
        ps_p = ctx.enter_context(tc.tile_pool(name="ps_p", bufs=2, space="PSUM"))
        ps_d = ctx.enter_context(tc.tile_pool(name="ps_d", bufs=1, space="PSUM"))

        # HAM warmup: the PE clock gate starts at half rate; a burst of
        # scratch matmuls (results discarded) unthrottles it before the
        # projection prologue.  scr is DVE-memset (DVE is otherwise idle
        # until the qtFull cast) so the GpSimd queue can issue DMAs first.
        scr = consts.tile([P, KJ], BF16, tag="scr")
        nc.vector.memset(scr[:], 0.5)
        dps = ps_d.tile([P, KJ], F32, tag="dps", name="dps_warm")

        def emit_dummy(dst):
            nc.tensor.matmul(
                dst[0:64, 0:KJ], scr[:, 0:64], scr[:, 0:KJ],
                start=True, stop=True,
            )

        for _ in range(30):
            emit_dummy(dps)

        # ---- loads ----
        # All input DMAs issue from the GpSimd queue (fast DGE dispatch,
        # engine otherwise idle).  wk/wq first (tiny, needed by the k-proj
        # as soon as sT chunk 0 lands), then all of sT (the critical path
        # to the first exp), then dT, then the remaining weights.
        wqt = consts.tile([P, CJ, F], BF16, tag="wq")
        wkt = consts.tile([P, CJ, F], BF16, tag="wk")
        wvst = consts.tile([P, CJ, F], BF16, tag="wvs")
        wvdt = consts.tile([P, CJ, F], BF16, tag="wvd")
        dq = nc.gpsimd if GPDMA else nc.scalar
        dqs = nc.gpsimd if GPDMA else nc.sync
        for w_ap, w_t in ((wk, wkt), (wq, wqt)):
            dq.dma_start(w_t[:], w_ap.rearrange("(co p) f -> p co f", p=P))
        sT3 = sT.rearrange("(co p) n -> p co n", p=P)
        dT3 = dT.rearrange("(co p) n -> p co n", p=P)
        sTc = []
        dTc = []
        for c in range(CJ):
            s_t = consts.tile([P, N], BF16, tag=f"sT{c}", name=f"sT{c}")
            dqs.dma_start(s_t[:], sT3[:, c])
            sTc.append(s_t)
        for c in range(CJ):
            d_t = consts.tile([P, N], BF16, tag=f"dT{c}", name=f"dT{c}")
            dqs.dma_start(d_t[:], dT3[:, c])
            dTc.append(d_t)
        for w_ap, w_t in ((wvs, wvst), (wvd, wvdt)):
            dq.dma_start(w_t[:], w_ap.rearrange("(co p) f -> p co f", p=P))
        wpst = consts.tile([P, C], BF16, tag="wps")
        wpdt = consts.tile([P, C], BF16, tag="wpd")
        dq.dma_start(wpst[:], wps)
        dq.dma_start(wpdt[:], wpd)

        # ---- q/k projections into transposed [feat, N] layout ----
        # QK runs as K=128 matmuls: per-head kT lives in a full-height
        # [128, N] tile with the other heads' feature rows zeroed (the zero
        # rows annihilate the cross-head products), keeping the PE activity
        # monitor from halving the clock on a K=32 contraction.
        qtFull = consts.tile([P, N], BF16, tag="qtFull")
        kT = consts.tile([P, N], BF16, tag="kT")
        # head 3 cannot use a K=32 partition slice (PE operands only accept
        # base partitions 0/32/64), so it keeps a zero-padded full-height
        # tile and a K=128 contraction like v2.
        kTz3 = consts.tile([P, N], BF16, tag="kTz3")
        ms = nc.gpsimd if GPMEMSET else nc.vector
        ms.memset(kTz3[:], 0.0)

        def emit_qproj(j):
            ps = ps_p.tile([P, KJ], F32, tag="pp", name="pp_q")
            for c in range(CJ):
                nc.tensor.matmul(
                    ps[:], wqt[:, c], sTc[c][:, ts(j, KJ)],
                    start=(c == 0), stop=(c == CJ - 1),
                )
            nc.vector.tensor_copy(qtFull[:, ts(j, KJ)], ps[:])

        # ---- value projections, natural [N, feat] layout, packed per head ----
        # vpk[n][:, h*VW:(h+1)*VW] = [v_se_h (32) | v_de_h (32) | ones (1)];
        # one tile per k-chunk so PV only depends on the chunks emitted so far.
        vpk = [consts.tile([P, HL * VW], BF16, tag=f"vpk{n}", name=f"vpk{n}")
               for n in range(NK)]
        for n in range(NK):   # denominator ones-columns, set once up front
            ms.memset(
                vpk[n].rearrange("p (h y) -> p h y", h=HL)[:, :, 2 * D:2 * D + 1], 1.0
            )

        def emit_vproj(n):
            # both branches into one [P, 2F] psum tile -> ONE strided DVE
            # copy into the per-head packed layout (halves the copy count)
            ps = ps_p.tile([P, KJ], F32, tag="pp", name="pp_v")
            for br, (act, w_t) in enumerate(((sTc, wvst), (dTc, wvdt))):
                for c in range(CJ):
                    nc.tensor.matmul(
                        ps[:, ds(br * F, F)], act[c][:, ts(n, P)], w_t[:, c],
                        start=(c == 0), stop=(c == CJ - 1),
                    )
            dst = vpk[n].rearrange("p (h y) -> p h y", h=HL)[:, :, 0:2 * D]
            dst = dst.rearrange("p h (br d) -> p h br d", br=2)
            src = ps[:, 0:2 * F].rearrange("p (br h d) -> p h br d", br=2, h=HL)
            nc.vector.tensor_copy(dst, src)

        # ---- attention ----
        outTs = consts.tile([P, N], BF16, tag="oTs")
        outTd = consts.tile([P, N], BF16, tag="oTd")

        ones64 = consts.tile([1, 2 * D], BF16)
        ms.memset(ones64[:], 1.0)
        jstate = {}

        jnorm = {}   # (j, h) -> (reciprocal row AP, opcF)

        def emit_norm_collect(j, h, op):
            """PV result [65, KJ] PSUM -> SBUF.  For j<3 the denominator row
            goes into the per-block collector so one approx-reciprocal
            serves all 4 heads; the LAST block runs a per-head
            reciprocal_approx_fast immediately so the epilogue never waits."""
            opcF = opool.tile([VW, KJ], F32, tag="opc", name=f"opc{h}")
            nc.vector.tensor_copy(opcF[:], op[:VW, :])
            if j < NJ - 1:
                if j not in jstate:
                    rb4_t = rpool.tile([P, KJ], F32, tag="rb4", name=f"rb4_{j}")
                    ms.memset(rb4_t[:], 1.0)
                    jstate[j] = (rb4_t, {})
                rb4, opcs = jstate[j]
                # DVE partition offsets must be 32-aligned: head h's
                # denominator lives at row h*D
                nc.vector.tensor_copy(
                    rb4[ds(h * D, 1), :], opcF[2 * D:2 * D + 1, :])
                opcs[h] = opcF
            else:
                if RECIPFAST:
                    # the custom-DVE op NaNs below ~32 partitions: run it
                    # over the whole [65, KJ] opcF (junk reciprocals in the
                    # v rows are never read) and keep the denominator row
                    rcp1 = rpool.tile([VW, KJ], F32, tag="rcp1", name=f"rcp1_{h}")
                    nc.vector.reciprocal_approx_fast(rcp1[:], opcF[:])
                else:
                    rcp1 = rpool.tile([1, KJ], F32, tag="rcp1", name=f"rcp1_{h}")
                    nc.vector.reciprocal(rcp1[:], opcF[2 * D:2 * D + 1, :])
                jnorm[(j, h)] = (rcp1[ds(2 * D, 1), :] if RECIPFAST
                                 else rcp1[0:1, :], opcF)

        def emit_norm_recip(j):
            """One approx reciprocal covers the block's 4 denominator rows
            (the unused rows hold 1.0 from the memset).  Consumers are
            deferred (emit_njob) so the PE queue never blocks on it."""
            rb4, opcs = jstate.pop(j)
            rcp4 = rpool.tile([P, KJ], F32, tag="rcp4")
            if RECIPFAST:
                nc.vector.reciprocal_approx_fast(rcp4[:], rb4[:])
            else:
                nc.vector.reciprocal(rcp4[:], rb4[:])
            for h in range(HL):
                jnorm[(j, h)] = (rcp4[ds(h * D, 1), :], opcs[h])

        def emit_njob(j, h):
            """Per-head normalize: broadcast the reciprocal across partitions
            via a ones-matmul (GPSIMD partition_broadcast ucode is not loaded
            on HW) and scale both branches into outTs/outTd."""
            rcp_row, opcF = jnorm.pop((j, h))
            rsb = rpool.tile([1, KJ], BF16, tag="rsb")
            nc.vector.tensor_copy(rsb[:], rcp_row)
            rb = ps_p.tile([2 * D, KJ], F32, tag="pp", name="pp_rb")
            nc.tensor.matmul(rb[:], ones64[:], rsb[:], start=True, stop=True)
            nc.vector.tensor_tensor(
                outTs[ds(h * D, D), ts(j, KJ)], opcF[0:D, :], rb[0:D, :], MUL)
            nc.vector.tensor_tensor(
                outTd[ds(h * D, D), ts(j, KJ)], opcF[D:2 * D, :], rb[D:2 * D, :], MUL)

        def emit_outproj_piece(j, nn):
            """One output chunk (both branches) — spread across iterations so
            the out-projection never blocks the QK stream for long."""
            for br, (oT, wp_t) in enumerate(((outTs, wpst), (outTd, wpdt))):
                pp = ps_p.tile([P, KJ], F32, tag="pp", name="pp_o")
                nc.tensor.matmul(
                    pp[:], oT[:, ds((NJ * j + nn) * P, P)], wp_t[:],
                    start=True, stop=True,
                )
                st = spool.tile([P, KJ], BF16, tag="st")
                nc.vector.tensor_copy(st[:], pp[:])
                nc.sync.dma_start(out[br][:, NJ * j + nn], st[:])

        # Prologue: k-projection (all blocks) + q-projection (block 0),
        # c-OUTER so each activation chunk is consumed as soon as its DMA
        # lands.  The k-proj accumulators live OUTSIDE the score banks
        # (ps_p x2 + ps_o + ps_d) so the first score matmuls only wait on
        # the qtFull / kTz(h0,j0) casts, not on draining all 16 kTz casts.
        kps = [
            ps_p.tile([P, KJ], F32, tag="pp", name="kps0"),
            ps_p.tile([P, KJ], F32, tag="pp", name="kps1"),
            ps_o.tile([P, KJ], F32, tag="op", name="kps2"),
            ps_d.tile([P, KJ], F32, tag="dps", name="kps3"),
        ]
        q0ps = ps_sc.tile([P, 2, KJ], F32, tag="sc", name="q0ps")
        for c in range(CJ):
            for j in range(NJ):
                nc.tensor.matmul(
                    kps[j][:], wkt[:, c], sTc[c][:, ts(j, KJ)],
                    start=(c == 0), stop=(c == CJ - 1),
                )
            nc.tensor.matmul(
                q0ps[:, 0], wqt[:, c], sTc[c][:, ts(0, KJ)],
                start=(c == 0), stop=(c == CJ - 1),
            )
        nc.vector.tensor_copy(qtFull[:, ts(0, KJ)], q0ps[:, 0])
        # kT casts are emitted LAZILY (one per slot): j0 first gates only
        # the first iteration's early groups; j3 (the ps_d bank) last.
        # Scores contract K=32 over head h's partition rows of kT/qtFull
        # directly -- no zero-padded per-head copies.  (The PE stream is
        # ~90% real work now, so the 25%-active score matmuls no longer
        # trip the HAM activity gate the way v2's K=32 attempt did.)
        cast_q = [0, 1, 2, 3]

        def emit_cast():
            j = cast_q.pop(0)
            nc.vector.tensor_copy(kT[:, ts(j, KJ)], kps[j][:])
            nc.vector.tensor_copy(
                kTz3[ds(3 * D, D), ts(j, KJ)], kps[j][ds(3 * D, D), :])

        emit_cast()  # j0: with qtFull this gates the first scores

        vq = list(range(NK))     # pending v-projection chunks
        ojobs = []               # pending out-projection pieces
        njobs = []               # pending per-head normalize chains
        pv_pending = deque()     # (j, h, g, op, pr) PV jobs trailing the exps
        slot = 0                 # global (j,h,g) slot counter

        def emit_pv_job():
            j, h, g, op, pr = pv_pending.popleft()
            for i in (0, 1):
                m = 2 * g + i
                nc.tensor.matmul(
                    op[:VW, :], vpk[m][:, ds(h * VW, VW)],
                    pr[:, i, :],
                    start=(m == 0), stop=(m == NK - 1),
                )
            if g == NG - 1:
                emit_norm_collect(j, h, op)
                if h == HL - 1 and j < NJ - 1:
                    emit_norm_recip(j)
                    njobs.extend((j, hh) for hh in range(HL))
                elif j == NJ - 1:
                    njobs.append((j, h))

        for j in range(NJ):
            for h in range(HL):
                op = ps_o.tile([P, KJ], F32, tag="op", name="op_pv")
                for g in range(NG):
                    # scores for k-chunks 2g, 2g+1 into a 2-bank psum tile,
                    # then one wide exp over both banks
                    sp = ps_sc.tile([P, 2, KJ], F32, tag="sc")
                    for i in (0, 1):
                        if h < HL - 1:
                            nc.tensor.matmul(
                                sp[:, i], kT[ds(h * D, D), ts(2 * g + i, P)],
                                qtFull[ds(h * D, D), ts(j, KJ)],
                                start=True, stop=True,
                            )
                        else:
                            nc.tensor.matmul(
                                sp[:, i], kTz3[:, ts(2 * g + i, P)],
                                qtFull[:, ts(j, KJ)], start=True, stop=True,
                            )
                    pr = ppool.tile([P, 2, KJ], BF16, tag="pr")
                    nc.scalar.activation(pr[:], sp[:], EXP, scale=SCALE)
                    pv_pending.append((j, h, g, op, pr))
                    slot += 1
                    # remaining kT casts, one per slot
                    if cast_q:
                        emit_cast()
                    # PV trails the exp stream by a FULL iteration (8
                    # groups).  The ACT queue then always holds ~8 exps of
                    # runway, so a PE stall (late DMA, PSUM WAR) of up to
                    # ~10us never gaps the exp stream; it also spreads each
                    # iteration's PV evenly across the next one, removing
                    # the h/j-boundary PV bunches of v2.
                    lag = 2 if (j, h) == (NJ - 1, HL - 1) else NG
                    while len(pv_pending) > lag:
                        emit_pv_job()
                    # v-projection fillers, once dT has had time to land and
                    # the k-proj accumulators have drained out of ps_p
                    if slot >= 7:
                        for _ in range(2):
                            if vq:
                                emit_vproj(vq.pop(0))
                    if j < NJ - 1 and h == 1 and g == 4:
                        # next block's q-projection — emitted away from the
                        # j-boundary so it never delays the boundary QK stream
                        emit_qproj(j + 1)
                    # deferred normalize chains and out-projection pieces
                    if g in (3, 4, 5, 6) and njobs:
                        jn, hn = njobs.pop(0)
                        emit_njob(jn, hn)
                        if hn == HL - 1:
                            ojobs += [(jn, nn) for nn in range(NJ)]
                    if g in (2, 5) and ojobs:
                        emit_outproj_piece(*ojobs.pop(0))
        # drain: remaining PV, normalize and out-projection jobs, interleaved
        # (the PV pops cascade norm_collect -> njobs -> ojobs), with scratch
        # matmuls keeping the HAM clock gate open through the tail
        dtail = ps_d.tile([P, KJ], F32, tag="dps", name="dps_tail")
        while pv_pending or njobs or ojobs:
            if pv_pending:
                emit_pv_job()
            if njobs:
                jn, hn = njobs.pop(0)
                emit_njob(jn, hn)
                if hn == HL - 1:
                    ojobs += [(jn, nn) for nn in range(NJ)]
            elif ojobs and not pv_pending:
                emit_outproj_piece(*ojobs.pop(0))
            emit_dummy(dtail)

    nc.compile()
    return nc


_NC_CACHE = {}


def _get_nc():
    if "nc" not in _NC_CACHE:
        _NC_CACHE["nc"] = build_nc()
    return _NC_CACHE["nc"]


def make_in_maps(se, de, W_qkv_se, W_v_de, W_proj_se, W_proj_de):
    se = np.asarray(se, dtype=np.float32)
    de = np.asarray(de, dtype=np.float32)
    W_qkv_se = np.asarray(W_qkv_se, dtype=np.float32)
    W_v_de = np.asarray(W_v_de, dtype=np.float32)
    W_proj_se = np.asarray(W_proj_se, dtype=np.float32)
    W_proj_de = np.asarray(W_proj_de, dtype=np.float32)
    qW, kW, vW = W_qkv_se[:, 0:C], W_qkv_se[:, C:2 * C], W_qkv_se[:, 2 * C:3 * C]

    sTs = [np.ascontiguousarray(se[b].T).astype(NPBF16) for b in range(B)]
    dTs = [np.ascontiguousarray(de[b].T).astype(NPBF16) for b in range(B)]
    in_maps = []
    for core in range(8):
        b, g = divmod(core, 4)
        sl = slice(g * F, (g + 1) * F)
        in_maps.append({
            "sT": sTs[b],
            "dT": dTs[b],
            "wq": np.ascontiguousarray(qW[:, sl]).astype(NPBF16),
            "wk": np.ascontiguousarray(kW[:, sl]).astype(NPBF16),
            "wvs": np.ascontiguousarray(vW[:, sl]).astype(NPBF16),
            "wvd": np.ascontiguousarray(W_v_de[:, sl]).astype(NPBF16),
            "wps": np.ascontiguousarray(W_proj_se[sl, :]).astype(NPBF16),
            "wpd": np.ascontiguousarray(W_proj_de[sl, :]).astype(NPBF16),
        })
    return in_maps


def gather_out(outs, b_proj_se, b_proj_de):
    b_proj_se = np.asarray(b_proj_se, dtype=np.float32)
    b_proj_de = np.asarray(b_proj_de, dtype=np.float32)
    # per-core out is packed [branch, partition, n-chunk, C] bf16 partials
    outs = [np.asarray(o).view(NPBF16).astype(np.float32)
            .transpose(0, 2, 1, 3).reshape(2, N, C)
            if np.asarray(o).dtype != np.float32 else
            np.asarray(o).transpose(0, 2, 1, 3).reshape(2, N, C)
            for o in outs]
    out_se = np.stack(
        [sum(outs[4 * b + g][0] for g in range(4)) for b in range(B)]
    ) + b_proj_se[None, None, :]
    out_de = np.stack(
        [sum(outs[4 * b + g][1] for g in range(4)) for b in range(B)]
    ) + b_proj_de[None, None, :]
    return out_se.astype(np.float32), out_de.astype(np.float32)


def kernel(se, de, W_qkv_se, W_v_de, W_proj_se, b_proj_se, W_proj_de, b_proj_de):
    nc = _get_nc()
    in_maps = make_in_maps(se, de, W_qkv_se, W_v_de, W_proj_se, W_proj_de)
    res = run_bass_kernel_spmd(nc, in_maps, core_ids=list(range(8)))
    outs = [r["out"] for r in res.results]
    return gather_out(outs, b_proj_se, b_proj_de)


# revision 15
# speedup vs baseline: 1.2827x; 1.2827x over previous
"""Dual-branch attention (shared attn weights, se/de value branches) on 8 TRN2 cores.

Sharding: 2 batches x 16 heads = 32 (b,h) pairs; core i owns batch i//4 and
heads [4*(i%4), 4*(i%4)+4) (128 feature channels). Activations are passed
pre-transposed ([C, N]) and in bf16 so the per-core kernel needs no on-chip
transposes. Each core computes its heads' attention for both value branches
and a row-sharded partial of the output projections; the host sums the 4
partials per batch and adds the biases.

v3 (this file), building on v2's wide-exp/deferred-normalize structure:
  - prologue: k-proj PSUM moved OFF the score banks (ps_p/ps_o/ps_d) so the
    first score matmuls only wait on the qtFull + kTz(h0,j0) casts; input
    DMAs are issued from the idle GpSimd queue (fast DGE dispatch) with sT
    before dT; kTz/vpk memsets run on GpSimd instead of clogging the DVE
    queue ahead of the kTz casts.  First exp ~39us -> ~18us.
  - reciprocals use the custom-DVE reciprocal_approx_fast (0.67us vs 4.3us;
    verified ~51 ULP on HW) and the last block's LN+EXP trick is gone.
  - PV matmuls ride a global pending queue lagging the exp stream by 2
    groups (6 during the first iteration so the v-projection, which needs
    the late-arriving dT chunks, never stalls the in-order PE queue), so
    h/j boundaries no longer bunch PV work between exps.
  - the 2-per-group scratch matmuls are dropped (PE is ~90% busy without
    them; HAM only needs the warmup burst + tail fillers).
"""

from collections import deque
from contextlib import ExitStack

import numpy as np
import ml_dtypes

import concourse.bass as bass
import concourse.mybir as mybir
import concourse.tile as tile
from concourse import bacc
from concourse.bass import ts, ds
from concourse.bass_utils import run_bass_kernel_spmd

B, N, C, H, D = 2, 2048, 512, 16, 32
SCALE = D ** -0.5
P = 128
CJ = C // P      # 4 contraction chunks for the projections
NJ = 4           # q blocks of 512
KJ = N // NJ     # 512
NK = N // P      # 16 k chunks of 128
NG = NK // 2     # 8 score groups of 2 k-chunks per (j,h)
HL = 4           # heads per core
F = HL * D       # 128 local feature channels
VW = 2 * D + 1   # per-head vpack width: [v_se | v_de | ones]

BF16 = mybir.dt.bfloat16
F32 = mybir.dt.float32
NPBF16 = ml_dtypes.bfloat16


def build_nc():
    import os
    GPDMA = os.environ.get("KRN_GPDMA", "1") == "1"
    GPMEMSET = os.environ.get("KRN_GPMEMSET", "1") == "1"
    RECIPFAST = os.environ.get("KRN_RECIPFAST", "1") == "1"
    nc = bacc.Bacc("TRN2", target_bir_lowering=False, debug=False, num_devices=8)

    sT = nc.dram_tensor("sT", [C, N], BF16, kind="ExternalInput").ap()
    dT = nc.dram_tensor("dT", [C, N], BF16, kind="ExternalInput").ap()
    wq = nc.dram_tensor("wq", [C, F], BF16, kind="ExternalInput").ap()
    wk = nc.dram_tensor("wk", [C, F], BF16, kind="ExternalInput").ap()
    wvs = nc.dram_tensor("wvs", [C, F], BF16, kind="ExternalInput").ap()
    wvd = nc.dram_tensor("wvd", [C, F], BF16, kind="ExternalInput").ap()
    wps = nc.dram_tensor("wps", [F, C], BF16, kind="ExternalInput").ap()
    wpd = nc.dram_tensor("wpd", [F, C], BF16, kind="ExternalInput").ap()
    # packed output: [branch, partition, n-chunk, C] bf16 partials; the host
    # transposes back to [branch, N, C] and sums partials in fp32.
    out = nc.dram_tensor("out", [2, P, NK, C], BF16, kind="ExternalOutput").ap()

    EXP = mybir.ActivationFunctionType.Exp
    MUL = mybir.AluOpType.mult

    with ExitStack() as ctx:
        tc = ctx.enter_context(tile.TileContext(nc))
        consts = ctx.enter_context(tc.tile_pool(name="consts", bufs=1))
        ppool = ctx.enter_context(tc.tile_pool(name="probs", bufs=12))
        opool = ctx.enter_context(tc.tile_pool(name="opool", bufs=6))
        rpool = ctx.enter_context(tc.tile_pool(name="rpool", bufs=8))
        spool = ctx.enter_context(tc.tile_pool(name="spool", bufs=4))
        ps_sc = ctx.enter_context(tc.tile_pool(name="ps_sc", bufs=2, space="PSUM"))
        ps_o = ctx.enter_context(tc.tile_pool(name="ps_o", bufs=1, space="PSUM"))
        ps_p = ctx.enter_context(tc.tile_pool(name="ps_p", bufs=2, space="PSUM"))
        ps_d = ctx.enter_context(tc.tile_pool(name="ps_d", bufs=1, space="PSUM"))

        # HAM warmup: the PE clock gate starts at half rate; a burst of
        # scratch matmuls (results discarded) unthrottles it before the
        # projection prologue.  scr is DVE-memset (DVE is otherwise idle
        # until the qtFull cast) so the GpSimd queue can issue DMAs first.
        scr = consts.tile([P, KJ], BF16, tag="scr")
        nc.vector.memset(scr[:], 0.5)
        dps = ps_d.tile([P, KJ], F32, tag="dps", name="dps_warm")

        def emit_dummy(dst):
            nc.tensor.matmul(
                dst[0:64, 0:KJ], scr[:, 0:64], scr[:, 0:KJ],
                start=True, stop=True,
            )

        for _ in range(30):
            emit_dummy(dps)

        # ---- loads ----
        # All input DMAs issue from the GpSimd queue (fast DGE dispatch,
        # engine otherwise idle).  wk/wq first (tiny, needed by the k-proj
        # as soon as sT chunk 0 lands), then all of sT (the critical path
        # to the first exp), then dT, then the remaining weights.
        wqt = consts.tile([P, CJ, F], BF16, tag="wq")
        wkt = consts.tile([P, CJ, F], BF16, tag="wk")
        wvst = consts.tile([P, CJ, F], BF16, tag="wvs")
        wvdt = consts.tile([P, CJ, F], BF16, tag="wvd")
        dq = nc.gpsimd if GPDMA else nc.scalar
        dqs = nc.gpsimd if GPDMA else nc.sync
        for w_ap, w_t in ((wk, wkt), (wq, wqt)):
            dq.dma_start(w_t[:], w_ap.rearrange("(co p) f -> p co f", p=P))
        sT3 = sT.rearrange("(co p) n -> p co n", p=P)
        dT3 = dT.rearrange("(co p) n -> p co n", p=P)
        sTc = []
        dTc = []
        for c in range(CJ):
            s_t = consts.tile([P, N], BF16, tag=f"sT{c}", name=f"sT{c}")
            dqs.dma_start(s_t[:], sT3[:, c])
            sTc.append(s_t)
        for c in range(CJ):
            d_t = consts.tile([P, N], BF16, tag=f"dT{c}", name=f"dT{c}")
            dqs.dma_start(d_t[:], dT3[:, c])
            dTc.append(d_t)
        for w_ap, w_t in ((wvs, wvst), (wvd, wvdt)):
            dq.dma_start(w_t[:], w_ap.rearrange("(co p) f -> p co f", p=P))
        wpst = consts.tile([P, C], BF16, tag="wps")
        wpdt = consts.tile([P, C], BF16, tag="wpd")
        dq.dma_start(wpst[:], wps)
        dq.dma_start(wpdt[:], wpd)

        # ---- q/k projections into transposed [feat, N] layout ----
        # QK runs as K=128 matmuls: per-head kT lives in a full-height
        # [128, N] tile with the other heads' feature rows zeroed (the zero
        # rows annihilate the cross-head products), keeping the PE activity
        # monitor from halving the clock on a K=32 contraction.
        qtFull = consts.tile([P, N], BF16, tag="qtFull")
        # per-head zero-padded kT tiles: K=32 contractions leave the PE
        # array 3/4 idle and the HAM activity monitor halves the clock
        # (re-measured: 234us vs 185us), so scores stay K=128 against
        # full-height tiles whose off-head rows are zero.
        kTz = [consts.tile([P, N], BF16, tag=f"kTz{h}", name=f"kTz{h}")
               for h in range(HL)]
        ms = nc.gpsimd if GPMEMSET else nc.vector
        for h in range(HL):
            ms.memset(kTz[h][:], 0.0)

        def emit_qproj(j):
            ps = ps_p.tile([P, KJ], F32, tag="pp", name="pp_q")
            for c in range(CJ):
                nc.tensor.matmul(
                    ps[:], wqt[:, c], sTc[c][:, ts(j, KJ)],
                    start=(c == 0), stop=(c == CJ - 1),
                )
            nc.vector.tensor_copy(qtFull[:, ts(j, KJ)], ps[:])

        # ---- value projections, natural [N, feat] layout, packed per head ----
        # vpk[n][:, h*VW:(h+1)*VW] = [v_se_h (32) | v_de_h (32) | ones (1)];
        # one tile per k-chunk so PV only depends on the chunks emitted so far.
        vpk = [consts.tile([P, HL * VW], BF16, tag=f"vpk{n}", name=f"vpk{n}")
               for n in range(NK)]
        for n in range(NK):   # denominator ones-columns, set once up front
            ms.memset(
                vpk[n].rearrange("p (h y) -> p h y", h=HL)[:, :, 2 * D:2 * D + 1], 1.0
            )

        def emit_vproj(n):
            # both branches into one [P, 2F] psum tile -> ONE strided DVE
            # copy into the per-head packed layout (halves the copy count)
            ps = ps_p.tile([P, KJ], F32, tag="pp", name="pp_v")
            for br, (act, w_t) in enumerate(((sTc, wvst), (dTc, wvdt))):
                for c in range(CJ):
                    nc.tensor.matmul(
                        ps[:, ds(br * F, F)], act[c][:, ts(n, P)], w_t[:, c],
                        start=(c == 0), stop=(c == CJ - 1),
                    )
            dst = vpk[n].rearrange("p (h y) -> p h y", h=HL)[:, :, 0:2 * D]
            dst = dst.rearrange("p h (br d) -> p h br d", br=2)
            src = ps[:, 0:2 * F].rearrange("p (br h d) -> p h br d", br=2, h=HL)
            nc.vector.tensor_copy(dst, src)

        # ---- attention ----
        outTs = consts.tile([P, N], BF16, tag="oTs")
        outTd = consts.tile([P, N], BF16, tag="oTd")

        ones64 = consts.tile([1, 2 * D], BF16)
        ms.memset(ones64[:], 1.0)
        jstate = {}

        jnorm = {}   # (j, h) -> (reciprocal row AP, opcF)

        def emit_norm_collect(j, h, op):
            """PV result [65, KJ] PSUM -> SBUF.  For j<3 the denominator row
            goes into the per-block collector so one approx-reciprocal
            serves all 4 heads; the LAST block runs a per-head
            reciprocal_approx_fast immediately so the epilogue never waits."""
            opcF = opool.tile([VW, KJ], F32, tag="opc", name=f"opc{h}")
            nc.vector.tensor_copy(opcF[:], op[:VW, :])
            if j < NJ - 1:
                if j not in jstate:
                    rb4_t = rpool.tile([P, KJ], F32, tag="rb4", name=f"rb4_{j}")
                    ms.memset(rb4_t[:], 1.0)
                    jstate[j] = (rb4_t, {})
                rb4, opcs = jstate[j]
                # DVE partition offsets must be 32-aligned: head h's
                # denominator lives at row h*D
                nc.vector.tensor_copy(
                    rb4[ds(h * D, 1), :], opcF[2 * D:2 * D + 1, :])
                opcs[h] = opcF
            else:
                if RECIPFAST:
                    # the custom-DVE op NaNs below ~32 partitions: run it
                    # over the whole [65, KJ] opcF (junk reciprocals in the
                    # v rows are never read) and keep the denominator row
                    rcp1 = rpool.tile([VW, KJ], F32, tag="rcp1", name=f"rcp1_{h}")
                    nc.vector.reciprocal_approx_fast(rcp1[:], opcF[:])
                else:
                    rcp1 = rpool.tile([1, KJ], F32, tag="rcp1", name=f"rcp1_{h}")
                    nc.vector.reciprocal(rcp1[:], opcF[2 * D:2 * D + 1, :])
                jnorm[(j, h)] = (rcp1[ds(2 * D, 1), :] if RECIPFAST
                                 else rcp1[0:1, :], opcF)

        def emit_norm_recip(j):
            """One approx reciprocal covers the block's 4 denominator rows
            (the unused rows hold 1.0 from the memset).  Consumers are
            deferred (emit_njob) so the PE queue never blocks on it."""
            rb4, opcs = jstate.pop(j)
            rcp4 = rpool.tile([P, KJ], F32, tag="rcp4")
            if RECIPFAST:
                nc.vector.reciprocal_approx_fast(rcp4[:], rb4[:])
            else:
                nc.vector.reciprocal(rcp4[:], rb4[:])
            for h in range(HL):
                jnorm[(j, h)] = (rcp4[ds(h * D, 1), :], opcs[h])

        def emit_njob(j, h):
            """Per-head normalize: broadcast the reciprocal across partitions
            via a ones-matmul (GPSIMD partition_broadcast ucode is not loaded
            on HW) and scale both branches into outTs/outTd."""
            rcp_row, opcF = jnorm.pop((j, h))
            rsb = rpool.tile([1, KJ], BF16, tag="rsb")
            nc.vector.tensor_copy(rsb[:], rcp_row)
            rb = ps_p.tile([2 * D, KJ], F32, tag="pp", name="pp_rb")
            nc.tensor.matmul(rb[:], ones64[:], rsb[:], start=True, stop=True)
            nc.vector.tensor_tensor(
                outTs[ds(h * D, D), ts(j, KJ)], opcF[0:D, :], rb[0:D, :], MUL)
            nc.vector.tensor_tensor(
                outTd[ds(h * D, D), ts(j, KJ)], opcF[D:2 * D, :], rb[D:2 * D, :], MUL)

        def emit_outproj_piece(j, nn):
            """One output chunk (both branches) — spread across iterations so
            the out-projection never blocks the QK stream for long."""
            for br, (oT, wp_t) in enumerate(((outTs, wpst), (outTd, wpdt))):
                pp = ps_p.tile([P, KJ], F32, tag="pp", name="pp_o")
                nc.tensor.matmul(
                    pp[:], oT[:, ds((NJ * j + nn) * P, P)], wp_t[:],
                    start=True, stop=True,
                )
                st = spool.tile([P, KJ], BF16, tag="st")
                nc.vector.tensor_copy(st[:], pp[:])
                nc.sync.dma_start(out[br][:, NJ * j + nn], st[:])

        # Prologue: k-projection (all blocks) + q-projection (block 0),
        # c-OUTER so each activation chunk is consumed as soon as its DMA
        # lands.  The k-proj accumulators live OUTSIDE the score banks
        # (ps_p x2 + ps_o + ps_d) so the first score matmuls only wait on
        # the qtFull / kTz(h0,j0) casts, not on draining all 16 kTz casts.
        kps = [
            ps_p.tile([P, KJ], F32, tag="pp", name="kps0"),
            ps_p.tile([P, KJ], F32, tag="pp", name="kps1"),
            ps_o.tile([P, KJ], F32, tag="op", name="kps2"),
            ps_d.tile([P, KJ], F32, tag="dps", name="kps3"),
        ]
        q0ps = ps_sc.tile([P, 2, KJ], F32, tag="sc", name="q0ps")
        for c in range(CJ):
            for j in range(NJ):
                nc.tensor.matmul(
                    kps[j][:], wkt[:, c], sTc[c][:, ts(j, KJ)],
                    start=(c == 0), stop=(c == CJ - 1),
                )
            nc.tensor.matmul(
                q0ps[:, 0], wqt[:, c], sTc[c][:, ts(0, KJ)],
                start=(c == 0), stop=(c == CJ - 1),
            )
        nc.vector.tensor_copy(qtFull[:, ts(0, KJ)], q0ps[:, 0])
        # kTz casts are emitted LAZILY (2 per slot) so the DVE queue isn't
        # 13us deep when the v-projection copies and opcF work arrive.
        # Order: (h0, all j) unblocks head 0's whole score stream; (h1,
        # j0/j1) + (h*, j2) free the first h1 groups / the ps_o (PV) bank;
        # ps_p (kps0/kps1) is fully freed by cast 13 for the v-projection.
        cast_q = [(0, 0), (0, 1), (0, 2), (0, 3),
                  (1, 0), (1, 1), (1, 2), (2, 2), (3, 2),
                  (2, 0), (2, 1), (3, 0), (3, 1),
                  (1, 3), (2, 3), (3, 3)]

        def emit_cast():
            h, j = cast_q.pop(0)
            nc.vector.tensor_copy(
                kTz[h][ds(h * D, D), ts(j, KJ)], kps[j][ds(h * D, D), :])

        emit_cast()  # (h0, j0): with qtFull this gates the first scores

        vq = list(range(NK))     # pending v-projection chunks
        ojobs = []               # pending out-projection pieces
        njobs = []               # pending per-head normalize chains
        pv_pending = deque()     # (j, h, g, op, pr) PV jobs trailing the exps
        slot = 0                 # global (j,h,g) slot counter

        def emit_pv_job():
            j, h, g, op, pr = pv_pending.popleft()
            for i in (0, 1):
                m = 2 * g + i
                nc.tensor.matmul(
                    op[:VW, :], vpk[m][:, ds(h * VW, VW)],
                    pr[:, i, :],
                    start=(m == 0), stop=(m == NK - 1),
                )
            if g == NG - 1:
                emit_norm_collect(j, h, op)
                if h == HL - 1 and j < NJ - 1:
                    emit_norm_recip(j)
                    njobs.extend((j, hh) for hh in range(HL))
                elif j == NJ - 1:
                    njobs.append((j, h))

        for j in range(NJ):
            for h in range(HL):
                op = ps_o.tile([P, KJ], F32, tag="op", name="op_pv")
                for g in range(NG):
                    # scores for k-chunks 2g, 2g+1 into a 2-bank psum tile,
                    # then one wide exp over both banks
                    sp = ps_sc.tile([P, 2, KJ], F32, tag="sc")
                    for i in (0, 1):
                        nc.tensor.matmul(
                            sp[:, i], kTz[h][:, ts(2 * g + i, P)],
                            qtFull[:, ts(j, KJ)], start=True, stop=True,
                        )
                    pr = ppool.tile([P, 2, KJ], BF16, tag="pr")
                    nc.scalar.activation(pr[:], sp[:], EXP, scale=SCALE)
                    pv_pending.append((j, h, g, op, pr))
                    slot += 1
                    # remaining kTz casts, 2 per slot
                    for _ in range(2):
                        if cast_q:
                            emit_cast()
                    # PV trails the exp stream by a FULL iteration (8
                    # groups).  The ACT queue then always holds ~8 exps of
                    # runway, so a PE stall (late DMA, PSUM WAR) of up to
                    # ~10us never gaps the exp stream; it also spreads each
                    # iteration's PV evenly across the next one, removing
                    # the h/j-boundary PV bunches of v2.
                    lag = 2 if (j, h) == (NJ - 1, HL - 1) else NG
                    while len(pv_pending) > lag:
                        emit_pv_job()
                    # v-projection fillers, once dT has had time to land and
                    # the k-proj accumulators have drained out of ps_p
                    if slot >= 7:
                        for _ in range(2):
                            if vq:
                                emit_vproj(vq.pop(0))
                    if j < NJ - 1 and h == 1 and g == 4:
                        # next block's q-projection — emitted away from the
                        # j-boundary so it never delays the boundary QK stream
                        emit_qproj(j + 1)
                    # deferred normalize chains and out-projection pieces
                    if g in (3, 4, 5, 6) and njobs:
                        jn, hn = njobs.pop(0)
                        emit_njob(jn, hn)
                        if hn == HL - 1:
                            ojobs += [(jn, nn) for nn in range(NJ)]
                    if g in (2, 5) and ojobs:
                        emit_outproj_piece(*ojobs.pop(0))
        # drain: remaining PV, normalize and out-projection jobs, interleaved
        # (the PV pops cascade norm_collect -> njobs -> ojobs), with scratch
        # matmuls keeping the HAM clock gate open through the tail
        dtail = ps_d.tile([P, KJ], F32, tag="dps", name="dps_tail")
        while pv_pending or njobs or ojobs:
            if pv_pending:
                emit_pv_job()
            if njobs:
                jn, hn = njobs.pop(0)
                emit_njob(jn, hn)
                if hn == HL - 1:
                    ojobs += [(jn, nn) for nn in range(NJ)]
            elif ojobs and not pv_pending:
                emit_outproj_piece(*ojobs.pop(0))
            emit_dummy(dtail)

    nc.compile()
    return nc


_NC_CACHE = {}


def _get_nc():
    if "nc" not in _NC_CACHE:
        _NC_CACHE["nc"] = build_nc()
    return _NC_CACHE["nc"]


def make_in_maps(se, de, W_qkv_se, W_v_de, W_proj_se, W_proj_de):
    se = np.asarray(se, dtype=np.float32)
    de = np.asarray(de, dtype=np.float32)
    W_qkv_se = np.asarray(W_qkv_se, dtype=np.float32)
    W_v_de = np.asarray(W_v_de, dtype=np.float32)
    W_proj_se = np.asarray(W_proj_se, dtype=np.float32)
    W_proj_de = np.asarray(W_proj_de, dtype=np.float32)
    qW, kW, vW = W_qkv_se[:, 0:C], W_qkv_se[:, C:2 * C], W_qkv_se[:, 2 * C:3 * C]

    sTs = [np.ascontiguousarray(se[b].T).astype(NPBF16) for b in range(B)]
    dTs = [np.ascontiguousarray(de[b].T).astype(NPBF16) for b in range(B)]
    in_maps = []
    for core in range(8):
        b, g = divmod(core, 4)
        sl = slice(g * F, (g + 1) * F)
        in_maps.append({
            "sT": sTs[b],
            "dT": dTs[b],
            "wq": np.ascontiguousarray(qW[:, sl]).astype(NPBF16),
            "wk": np.ascontiguousarray(kW[:, sl]).astype(NPBF16),
            "wvs": np.ascontiguousarray(vW[:, sl]).astype(NPBF16),
            "wvd": np.ascontiguousarray(W_v_de[:, sl]).astype(NPBF16),
            "wps": np.ascontiguousarray(W_proj_se[sl, :]).astype(NPBF16),
            "wpd": np.ascontiguousarray(W_proj_de[sl, :]).astype(NPBF16),
        })
    return in_maps


def gather_out(outs, b_proj_se, b_proj_de):
    b_proj_se = np.asarray(b_proj_se, dtype=np.float32)
    b_proj_de = np.asarray(b_proj_de, dtype=np.float32)
    # per-core out is packed [branch, partition, n-chunk, C] bf16 partials
    outs = [np.asarray(o).view(NPBF16).astype(np.float32)
            .transpose(0, 2, 1, 3).reshape(2, N, C)
            if np.asarray(o).dtype != np.float32 else
            np.asarray(o).transpose(0, 2, 1, 3).reshape(2, N, C)
            for o in outs]
    out_se = np.stack(
        [sum(outs[4 * b + g][0] for g in range(4)) for b in range(B)]
    ) + b_proj_se[None, None, :]
    out_de = np.stack(
        [sum(outs[4 * b + g][1] for g in range(4)) for b in range(B)]
    ) + b_proj_de[None, None, :]
    return out_se.astype(np.float32), out_de.astype(np.float32)


def kernel(se, de, W_qkv_se, W_v_de, W_proj_se, b_proj_se, W_proj_de, b_proj_de):
    nc = _get_nc()
    in_maps = make_in_maps(se, de, W_qkv_se, W_v_de, W_proj_se, W_proj_de)
    res = run_bass_kernel_spmd(nc, in_maps, core_ids=list(range(8)))
    outs = [r["out"] for r in res.results]
    return gather_out(outs, b_proj_se, b_proj_de)
